# revision 6
# baseline (speedup 1.0000x reference)
"""DRNN-Char (4-layer dilated QRNN + decoder) Trainium2 kernel, v2.

Sharding: data-parallel over batch. 16 rows / 8 cores = 2 rows per core.

Key design points vs v1:
- Layer 0 is pure table lookup: every gate value is a function of the input
  token only, so the host precomputes z'0=(1-f0)*tanh(z0), f0, sigmoid(o0)
  per (vocab, unit) and sends them gathered per token. No L0 matmul, no L0
  activations on device.
- Direct C-space recurrence c = f*c + (1-f)*tanh(z) (scan carry is fp32
  internally regardless of operand dtype), so every elementwise tensor is
  bf16 and zero-centered: DVE runs at 2x, no sign/offset folding anywhere.
  Tanh and Sigmoid live in the same activation table set (no reloads).
- f,o gate matmuls run in fp8e4 with MatmulPerfMode.DoubleRow (2 rows of
  the contraction per cycle = 2x PE throughput). The z gate stays bf16
  (z errors are first-order in the output; f,o only second-order).
  Scales: weights *32, activations *SX[layer] to sit in e4m3 range; the
  activation instruction divides them back out.
- Each dilated layer stores its activations in "dilation order" (all t=j mod
  rate contiguous), so DVE scans are contiguous (stride-1, 2x mode). The
  inter-layer reorder rides on the matmul rhs access pattern (stride-2
  column reads).
- GpSimd takes the fp8 x-copy (combine2) and decoder PSUM copies; Scalar
  does 2-bank [128,1024] activations.
"""

import numpy as np
import ml_dtypes

EMB = 256
HID = 512
LAYERS = 4
VOCAB = 256
B = 16
T = 2048
NCORES = 8
BC = B // NCORES
HCH = HID // 128

SW = 32.0                      # fp8 weight scale
SX = [32.0, 128.0, 256.0]      # fp8 x scale for h0,h1,h2 (inputs of L1..L3)

_cache = {}


def _build():
    if "nc" in _cache:
        return _cache["nc"]

    import concourse.bass as bass
    import concourse.mybir as mybir
    import concourse.tile as tile
    from concourse import bacc

    f32 = mybir.dt.float32
    bf16 = mybir.dt.bfloat16
    fp8 = mybir.dt.float8e4
    SIG = mybir.ActivationFunctionType.Sigmoid
    TANH = mybir.ActivationFunctionType.Tanh
    MULT = mybir.AluOpType.mult
    ADD = mybir.AluOpType.add
    SUB = mybir.AluOpType.subtract
    DR = mybir.MatmulPerfMode.DoubleRow

    nc = bacc.Bacc(
        "TRN2",
        target_bir_lowering=False,
        debug=False,
        enable_asserts=False,
        num_devices=NCORES,
    )

    # ---- DRAM inputs (host-prepped, per core) ----
    zp0_d = nc.dram_tensor("zp0", [BC, 4, 128, 4, 512], bf16, kind="ExternalInput").ap()
    f0_d = nc.dram_tensor("f0", [BC, 4, 128, 4, 512], bf16, kind="ExternalInput").ap()
    so0_d = nc.dram_tensor("so0", [BC, 4, 128, 4, 512], bf16, kind="ExternalInput").ap()
    wz_d = nc.dram_tensor("wz", [3, 128, 4, 512], bf16, kind="ExternalInput").ap()
    wfo_d = nc.dram_tensor("wfo", [3, 128, 4, 1024], fp8, kind="ExternalInput").ap()
    wd_d = nc.dram_tensor("wd", [128, 4, VOCAB], bf16, kind="ExternalInput").ap()
    bias_d = nc.dram_tensor("bias", [128, 3, 12], f32, kind="ExternalInput").ap()
    decb_d = nc.dram_tensor("decb", [1, VOCAB], bf16, kind="ExternalInput").ap()
    out_d = nc.dram_tensor("out", [BC, T, VOCAB], f32, kind="ExternalOutput").ap()

    with tile.TileContext(nc) as tc:
        with (
            tc.tile_pool(name="consts", bufs=1) as consts,
            tc.tile_pool(name="acts", bufs=1) as acts,
            tc.tile_pool(name="l0t", bufs=2) as l0t,
            tc.tile_pool(name="stage", bufs=2) as stage,
            tc.tile_pool(name="ccl0", bufs=1) as ccl0,
            tc.tile_pool(name="ccp", bufs=2) as ccp,
            tc.tile_pool(name="outs", bufs=4) as outs,
            tc.tile_pool(name="psum", bufs=4, space="PSUM") as psum,
        ):
            # ---- resident tiles ----
            wz_sb = [consts.tile([128, 4, 512], bf16, tag=f"wz{i}", name=f"wz{i}") for i in range(3)]
            wfo_sb = [consts.tile([128, 4, 1024], fp8, tag=f"wfo{i}", name=f"wfo{i}") for i in range(3)]
            wd = consts.tile([128, 4, VOCAB], bf16, tag="wd", name="wd")
            bias = consts.tile([128, 3, 12], f32, tag="bias", name="bias")
            decb = consts.tile([1, VOCAB], bf16, tag="decb", name="decb")
            ones = consts.tile([1, 128], bf16, tag="ones", name="ones")

            xbuf = [acts.tile([128, 4, T], bf16, tag=f"x{r}", name=f"x{r}") for r in range(BC)]
            hbuf = [acts.tile([128, 4, T], bf16, tag=f"h{r}", name=f"h{r}") for r in range(BC)]
            x8 = [acts.tile([128, 4, T], fp8, tag=f"x8{r}", name=f"x8{r}") for r in range(BC)]

            # ---- const DMAs (gpsimd queue; tables go on sync queue) ----
            for i in range(3):
                nc.gpsimd.dma_start(wz_sb[i][:], wz_d[i])
                nc.gpsimd.dma_start(wfo_sb[i][:], wfo_d[i])
            nc.gpsimd.dma_start(wd[:], wd_d[:])
            nc.gpsimd.dma_start(bias[:], bias_d[:])
            nc.gpsimd.dma_start(decb[:], decb_d[:])
            nc.gpsimd.memset(ones[:], 1.0)

            # ---- layer 0: scan over host-gathered tables ----
            for r in range(BC):
                cc0 = [ccl0.tile([128, T], bf16, tag=f"cc0_{h}", name=f"cc0_{h}") for h in range(HCH)]
                for q4 in range(4):
                    zp = l0t.tile([128, 4, 512], bf16, tag="zp", name="zp")
                    f0t = l0t.tile([128, 4, 512], bf16, tag="f0", name="f0")
                    so0t = l0t.tile([128, 4, 512], bf16, tag="so0", name="so0")
                    nc.sync.dma_start(zp[:], zp0_d[r, q4])
                    nc.sync.dma_start(f0t[:], f0_d[r, q4])
                    nc.sync.dma_start(so0t[:], so0_d[r, q4])
                    sl = slice(q4 * 512, q4 * 512 + 512)
                    for h in range(HCH):
                        init = 0.0 if q4 == 0 else cc0[h][:, q4 * 512 - 1 : q4 * 512]
                        nc.vector.tensor_tensor_scan(
                            cc0[h][:, sl], f0t[:, h, :], zp[:, h, :],
                            initial=init, op0=MULT, op1=ADD,
                        )
                        nc.vector.tensor_tensor(
                            xbuf[r][:, h, sl], so0t[:, h, :], cc0[h][:, sl], MULT
                        )
                        nc.vector.scalar_tensor_tensor(
                            x8[r][:, h, sl], so0t[:, h, :], SX[0], cc0[h][:, sl],
                            MULT, MULT,
                        )

            # ---- layers 1..3 ----
            for li in (1, 2, 3):
                idx = li - 1
                rho = 2 ** li
                NC = min(512, T // rho)
                # x (previous layer order) column offset for this layer's chunk c:
                # source index = off + 2*n, n in [0, NC)
                if li in (1, 2):
                    offs = [0, 1024, 1, 1025]
                else:
                    offs = [0, 512, 1024, 1536, 1, 513, 1025, 1537]
                PER = 1024 // NC  # chunks per 2-bank psum tile
                ascale = 1.0 / (SW * SX[idx])
                for r in range(BC):
                    xin, hout, x8in = xbuf[r], hbuf[r], x8[r]
                    for h in range(HCH):
                        gts = {}
                        # z gate: bf16 matmuls, tanh(-1 * psum - bz) = -tanh(pre)...
                        # NOTE: scale=-1 gives tanh(-pre - bz)? We want
                        # ztneg = tanh(-(pre + bz)): bias must be -bz, scale -1.
                        zt = stage.tile([128, T], bf16, tag="zt", name="zt")
                        for pb in range(2):
                            ps = psum.tile([128, 1024], f32, tag="ps", name="ps")
                            for u in range(PER):
                                c = pb * PER + u
                                for k in range(4):
                                    nc.tensor.matmul(
                                        ps[:, u * NC : (u + 1) * NC],
                                        lhsT=wz_sb[idx][:, k, h * 128 : (h + 1) * 128],
                                        rhs=xin[:, k, offs[c] : offs[c] + 2 * NC - 1 : 2],
                                        start=(k == 0),
                                        stop=(k == 3),
                                    )
                            nc.scalar.activation(
                                zt[:, pb * 1024 : (pb + 1) * 1024], ps[:], TANH,
                                bias=bias[:, idx, h : h + 1], scale=-1.0,
                            )
                        gts["z"] = zt
                        # f,o gates: fp8 DoubleRow matmuls
                        for g, gname in ((0, "f"), (1, "o")):
                            gt = stage.tile([128, T], bf16, tag=gname, name=gname)
                            for pb in range(2):
                                ps = psum.tile([128, 1024], f32, tag="ps", name="ps")
                                for u in range(PER):
                                    c = pb * PER + u
                                    for kp in range(2):
                                        nc.tensor.matmul(
                                            ps[:, u * NC : (u + 1) * NC],
                                            lhsT=wfo_sb[idx][:, 2 * kp : 2 * kp + 2, g * 512 + h * 128 : g * 512 + (h + 1) * 128],
                                            rhs=x8in[:, 2 * kp : 2 * kp + 2, offs[c] : offs[c] + 2 * NC - 1 : 2],
                                            start=(kp == 0),
                                            stop=(kp == 1),
                                            perf_mode=DR,
                                        )
                                nc.scalar.activation(
                                    gt[:, pb * 1024 : (pb + 1) * 1024], ps[:], SIG,
                                    bias=bias[:, idx, (g + 1) * 4 + h : (g + 1) * 4 + h + 1],
                                    scale=ascale,
                                )
                            gts[gname] = gt
                        # z' = (f - 1) * (-tanh(z)) = (1-f) tanh(z), in place over zt
                        nc.vector.scalar_tensor_tensor(
                            gts["z"][:], gts["f"][:], 1.0, gts["z"][:], SUB, MULT
                        )
                        cc = ccp.tile([128, T], bf16, tag="cc", name="cc")
                        for j in range(rho):
                            ssl = slice(j * (T // rho), (j + 1) * (T // rho))
                            nc.vector.tensor_tensor_scan(
                                cc[:, ssl], gts["f"][:, ssl], gts["z"][:, ssl],
                                initial=0.0, op0=MULT, op1=ADD,
                            )
                        nc.vector.tensor_tensor(hout[:, h, :], gts["o"][:], cc[:], MULT)
                        if li < 3:
                            nc.vector.scalar_tensor_tensor(
                                x8in[:, h, :], gts["o"][:], SX[li], cc[:], MULT, MULT
                            )
                    xbuf[r], hbuf[r] = hbuf[r], xbuf[r]

            # ---- decoder (h3 is in dilation-8 order; scatter rows on DMA out) ----
            for r in range(BC):
                xin = xbuf[r]
                for mt in range(T // 128):
                    ps = psum.tile([128, 1024], f32, tag="ps", name="ps")
                    for k in range(4):
                        nc.tensor.matmul(
                            ps[:, 0:VOCAB],
                            lhsT=xin[:, k, mt * 128 : (mt + 1) * 128],
                            rhs=wd[:, k, :],
                            start=(k == 0),
                            stop=False,
                        )
                    nc.tensor.matmul(
                        ps[:, 0:VOCAB], lhsT=ones[:], rhs=decb[:],
                        start=False, stop=True,
                    )
                    ot = outs.tile([128, VOCAB], f32, tag="ot", name="ot")
                    nc.scalar.copy(ot[:], ps[:, 0:VOCAB])
                    # dilation-8 index i = j*256 + q -> t = 8q + j
                    t0 = 1024 * (mt % 2) + mt // 2
                    nc.sync.dma_start(out_d[r, t0 : t0 + 1017 : 8, :], ot[:])

    nc.compile()
    _cache["nc"] = nc
    return nc


def _prep_inputs(inputs):
    bf = ml_dtypes.bfloat16
    f8 = ml_dtypes.float8_e4m3fn
    x = np.asarray(inputs["x"]).astype(np.int64)
    emb = np.asarray(inputs["emb"], dtype=np.float32)
    Ws = [np.asarray(inputs[f"W{i}"], dtype=np.float32) for i in range(LAYERS)]
    bs = [np.asarray(inputs[f"b{i}"], dtype=np.float32) for i in range(LAYERS)]
    decW = np.asarray(inputs["decW"], dtype=np.float32)
    decb = np.asarray(inputs["decb"], dtype=np.float32)

    # layer-0 per-vocab gate tables
    pre0 = emb @ Ws[0] + bs[0]          # [VOCAB, 3H]
    zt0 = np.tanh(pre0[:, :HID])
    f0 = 1.0 / (1.0 + np.exp(-pre0[:, HID : 2 * HID]))
    so0 = 1.0 / (1.0 + np.exp(-pre0[:, 2 * HID :]))
    f0b = f0.astype(bf)
    zp0 = ((1.0 - f0b.astype(np.float32)) * zt0).astype(bf)
    so0b = so0.astype(bf)

    def table_arrange(tab, idx):
        # tab [VOCAB, HID] -> gathered [T, HID] -> [4(q), 128, 4(k), 512]
        g = tab[idx]                                  # [T, 512]
        return np.ascontiguousarray(
            g.T.reshape(4, 128, 4, 512).transpose(2, 1, 0, 3)
        )

    wz = np.stack(
        [np.ascontiguousarray(Ws[i][:, :HID].reshape(4, 128, 512).transpose(1, 0, 2)).astype(bf) for i in range(1, 4)]
    )
    wfo = np.stack(
        [
            np.ascontiguousarray((Ws[i][:, HID:] * SW).reshape(4, 128, 1024).transpose(1, 0, 2)).astype(f8)
            for i in range(1, 4)
        ]
    )
    wdt = np.ascontiguousarray(decW.reshape(4, 128, VOCAB).transpose(1, 0, 2)).astype(bf)

    bias = np.zeros((128, 3, 12), np.float32)
    for i in range(1, 4):
        bb = bs[i].reshape(3, 4, 128)  # [gate, h, p]
        bias[:, i - 1, 0:4] = -bb[0].T
        bias[:, i - 1, 4:8] = bb[1].T
        bias[:, i - 1, 8:12] = bb[2].T

    decbb = decb.reshape(1, VOCAB).astype(bf)

    in_maps = []
    for c in range(NCORES):
        zp_r = np.stack([table_arrange(zp0, x[BC * c + r]) for r in range(BC)])
        f_r = np.stack([table_arrange(f0b, x[BC * c + r]) for r in range(BC)])
        so_r = np.stack([table_arrange(so0b, x[BC * c + r]) for r in range(BC)])
        in_maps.append(
            {
                "zp0": zp_r,
                "f0": f_r,
                "so0": so_r,
                "wz": wz,
                "wfo": wfo,
                "wd": wdt,
                "bias": bias,
                "decb": decbb,
            }
        )
    return in_maps


def _unpermute(res):
    # rows come back in t-order already (DMA scatter); just concat cores
    out = np.empty((B, T, VOCAB), np.float32)
    for c in range(NCORES):
        out[BC * c : BC * (c + 1)] = res[c]["out"]
    return out


def kernel(**inputs) -> np.ndarray:
    from concourse.bass_utils import run_bass_kernel_spmd

    try:
        import jax, tempfile, os

        jax.config.update(
            "jax_compilation_cache_dir",
            os.environ.get("JAX_COMPILATION_CACHE_DIR")
            or os.path.join(tempfile.gettempdir(), "bass_jax_cache"),
        )
    except Exception:
        pass

    nc = _build()
    in_maps = _prep_inputs(inputs)
    res = run_bass_kernel_spmd(nc, in_maps, list(range(NCORES)))
    return _unpermute(res.results)
